# revision 35
# baseline (speedup 1.0000x reference)
"""Binary complex conv (BC conv) on 8 TRN2 NeuronCores.

Reference computation:
    xb = sign(x + 1e-6)                      # (16, 256, 112, 112)
    wr = sign(weight_real + 1e-6)            # (128, 128, 3, 3)
    wi = sign(weight_imag + 1e-6)
    kernel = [[wr, -wi], [wi, wr]]           # (256, 256, 3, 3)
    out = conv2d(xb, kernel, pad=1) + bias   # (16, 256, 112, 112)

Strategy: pure data-parallel over batch (2 images per core); everything
else on-device, numerically exact (all matmul operands are +-1/0/+-2 ->
exact in fp8e4; PSUM accumulates fp32; fp16 output is exact for the
integer conv part, bias rounds at ~2^-10).

Two tricks on top of the direct conv:
 * Karatsuba for the complex structure: A = xr*wr, B = xi*wi,
   C = (xr+xi)*(wr+wi); out_real = A-B, out_imag = C-A-B.
   3 convs of 128 input channels instead of 4.
 * fp8 DoubleRow: each binarized frame is stored with row stride 113
   (shared pad column: each row's right pad is the next row's left
   pad); conv taps in raster order have flat offsets [-114,-113,-112,
   -1,0,1,112,113,114], so consecutive taps pair into DoubleRow
   matmuls (contraction 256) + one normal matmul per conv.
   (Measured: every matmul streams at ~194ns = the 456-column output
   rate, so the steady state IS the PE streaming bound; DoublePixel
   is uint8-only in silicon and walrus rejects integer matmuls.)

Each 4-output-row tile accumulates into a [128, 452] PSUM bank
(garbage pad lanes skipped on eviction).

I/O transport (all actual computation stays on device):
 * x and weights ship as fp8e5 raw casts of the fp32 inputs - sign(x +
   1e-6) is preserved for all but ~3e-6 of elements (measured 159 of
   51M, zero weight flips), far inside the 2e-2 tolerance. 4x less
   input DMA; the bandwidth-bound head shrinks accordingly.
 * fp16 output, shipped as flat 448-element runs: halves output HBM
   traffic and the end-of-kernel drain.

Scheduling (from trace analysis):
 * Sign act-table preloaded at t=0; weight Signs split in piece-gated
   chunks so the first conv's DoubleRow matmuls start on taps 0-5
   while taps 6-8 still binarize.
 * head DMAs batched into few issues (each DMA_DIRECT2D costs ~650ns
   of queue occupancy, which otherwise dominates the head): one
   weights+bias DMA, one 14-row r-half, one 14-row i-half.
 * binarize in 14-row units (one DMA, both halves), Signs in 3/4-row
   chunks so no engine queue holds a long op ahead of the short PSUM
   evacuations that gate bank recycling; one unit per 3 tiles.
 * engine balance: ScalarE: Sign binarize + A-bank evac; DVE: B/C-bank
   evac + imag assembly + 3/7 of sums; GpSimd: real assembly + 4/7 of
   sums. In the last band (no more binarize work) Bn0 moves to ScalarE
   so DVE+GpSimd enter the tail without backlog; the last tiles' imag
   assembly and output DMAs split in 2-row chunks to shorten the
   final serial chain.
"""

import numpy as np

import concourse.bass as bass
import concourse.tile as tile
from concourse import mybir
from concourse.bass_utils import run_bass_kernel_spmd

N_CORES = 8
B = 16
CPB = 128          # channels per block (partition dim)
H = W = 112
RS = 113           # frame row stride (shared pad column:
                   # col 112 of row y is col 0 of row y+1)
FROWS = 116        # 114 padded rows + 2 junk margin rows
IMGS = 2
TROWS = 4          # output rows per matmul tile
NT = TROWS * RS    # matmul free dim (456)
BAND = 28
UR = 7             # binarize unit rows
EPS = 1e-6
WCOLS = 9 * CPB    # 1152
WTOT = 2 * WCOLS      # wr | wi

F32 = mybir.dt.float32
F16 = mybir.dt.float16
FP8 = mybir.dt.float8e4
FP8E5 = mybir.dt.float8e5
AF = mybir.ActivationFunctionType
DRM = mybir.MatmulPerfMode.DoubleRow
ALU = mybir.AluOpType

# tap flat offsets in raster order; pairs (0,1) (2,3) (4,5) (6,7), single 8
TAP_OFF = [dy * RS + dx for dy in (-1, 0, 1) for dx in (-1, 0, 1)]

# sign-chunk row ranges within a 7-row unit
SIGN_CHUNKS = ((0, 4), (4, 7))


def _split_multiwait(nc):
    """Walrus in this container rejects >1 semaphore wait per instruction
    ("Too many sync wait commands"); hoist extra waits onto preceding nops
    on the same engine."""
    import bass_rust

    for f in nc.m.functions:
        for bb in f.blocks:
            new_insts = []
            for inst in bb.instructions:
                si = inst.sync_info
                waits = list(si.on_wait) if si is not None and si.on_wait else []
                if len(waits) > 1:
                    for w in waits[:-1]:
                        nop = mybir.InstNoOp(
                            name=nc.get_next_instruction_name(),
                            engine=inst.engine,
                            ins=[],
                            outs=[],
                        )
                        nop.sync_info = bass_rust.SyncInfo(on_wait=[w], on_update=[])
                        new_insts.append(nop)
                    si.on_wait = [waits[-1]]
                    inst.sync_info = si
                new_insts.append(inst)
            bb.instructions = new_insts


def build_nc():
    nc = bass.Bass()

    x_ext = nc.declare_dram_parameter("x", [IMGS, 2 * CPB, H, W], FP8E5,
                                      isOutput=False)
    w_ext = nc.declare_dram_parameter("wT", [CPB, WTOT], FP8E5, isOutput=False)
    b_ext = nc.declare_dram_parameter("bias2", [CPB, 2], F32, isOutput=False)
    out_ext = nc.declare_dram_parameter("out", [IMGS, 2 * CPB, H, W], F16,
                                        isOutput=True)

    x_flat = x_ext.rearrange("b c h w -> (b c) (h w)")     # [512, 12544]
    out_flat = out_ext.rearrange("b c h w -> (b c) (h w)")

    with tile.TileContext(nc) as tc:
        with (
            tc.tile_pool(name="wstage", bufs=1) as wstage_pool,
            tc.tile_pool(name="wbin", bufs=1) as wbin_pool,
            tc.tile_pool(name="biasp", bufs=1) as bias_pool,
            tc.tile_pool(name="xq", bufs=1) as xq_pool,
            tc.tile_pool(name="stage", bufs=6) as stage_pool,
            tc.tile_pool(name="tmp", bufs=9) as tmp_pool,
            tc.tile_pool(name="outsb", bufs=8) as out_pool,
            tc.tile_pool(name="psum", bufs=8, space="PSUM") as psum_pool,
        ):
            # per-partition scalar constant for activation bias
            eps_pos = bias_pool.tile([CPB, 1], F32, tag="epsp")
            nc.gpsimd.memset(eps_pos[:], EPS)

            # HAM warmup: dummy matmuls on junk data with no dependencies so
            # the PE clock-gate reaches 8/8 before the first real matmul.
            # Sized to end roughly when the first conv's inputs land.
            junk = bias_pool.tile([CPB, 512], FP8, tag="junk")
            nc.gpsimd.memset(junk[:, 0:1], 1.0)

            # Sign act-table preload: a tiny dependency-free activation so
            # the 1.3us ACT_TABLE_LOAD runs at t~0, not ahead of the first
            # real binarize.
            sgate = bias_pool.tile([CPB, 1], FP8, tag="sgate")
            nc.scalar.activation(sgate[:], eps_pos[:], AF.Sign,
                                 bias=eps_pos[:], scale=1.0)

            jps = psum_pool.tile([CPB, 512], F32, tag="ps", name="jps")
            for _ in range(8):
                nc.tensor.matmul(jps[:], junk[:, :CPB], junk[:], start=True,
                                 stop=True)
            for _ in range(6):
                nc.tensor.matmul(jps[:, :256], junk[:, :CPB], junk[:, :256],
                                 start=True, stop=True)
            jout = bias_pool.tile([CPB, 1], F32, tag="jout")
            nc.vector.tensor_copy(jout[:], jps[:, 0:1])

            # DMA wake: a 128B transfer ahead of the weights so the first
            # real DMA doesn't pay the cold-path latency
            wake = bias_pool.tile([CPB, 1], FP8E5, tag="wake")
            nc.sync.dma_start(wake[:], x_flat[0:CPB, 0:1])

            # ---- weights: one DMA for wr|wi|bias ----
            w_sb = wstage_pool.tile([CPB, WTOT], FP8E5, tag="wstage")
            nc.sync.dma_start(w_sb[:], w_ext[:])

            # binarized fp8 weights [ci, tap, co]; wq_s = wq_r + wq_i
            wq_r = wbin_pool.tile([CPB, 9, CPB], FP8, tag="wqr")
            wq_i = wbin_pool.tile([CPB, 9, CPB], FP8, tag="wqi")
            wq_s = wbin_pool.tile([CPB, 9, CPB], FP8, tag="wqs")
            wq_rf = wq_r[:].rearrange("p t c -> p (t c)")
            wq_if = wq_i[:].rearrange("p t c -> p (t c)")

            def wsign(dstf, tmpf, col0, a, b):
                # DVE binarize: (w >= -eps)*2 - 1 in {-1,1}; keeps the
                # weight binarize off the ScalarE queue, which the unit
                # Signs and PSUM evacs saturate in the head
                nc.vector.tensor_scalar(tmpf[:, a:b], w_sb[:, col0 + a:col0 + b],
                                        -EPS, 2.0, op0=ALU.is_ge, op1=ALU.mult)
                nc.vector.tensor_scalar(dstf[:, a:b], tmpf[:, a:b],
                                        1.0, None, op0=ALU.subtract)

            # ---- persistent binarized fp8 frames ----
            # frame: [128, FROWS, RS]; frame row = padded row + 1 (1 junk
            # margin row on top); cols 0 / 113 are the zero pad columns,
            # cols 114-115 slack (only ever read into discarded pad lanes)
            def frame(nm):
                return xq_pool.tile([CPB, FROWS, RS], FP8, tag=nm, name=nm)

            xqr = [frame(f"xqr{i}") for i in range(IMGS)]
            xqi = [frame(f"xqi{i}") for i in range(IMGS)]
            xqs = [frame(f"xqs{i}") for i in range(IMGS)]

            def frame_memsets(i):
                eng = nc.vector if i == 0 else nc.gpsimd
                for t in (xqr[i], xqi[i], xqs[i]):
                    eng.memset(t[:, 1:2, :], 0.0)          # padded row 0
                    eng.memset(t[:, 114:115, :], 0.0)      # padded row 113
                    eng.memset(t[:, 1:116, 0:1], 0.0)      # padded col 0 (+row
                   # 115's, read as row 114's shared right pad)

            frame_memsets(0)  # img1's memsets deferred past the first band

            flat = {}
            for i in range(IMGS):
                flat[("r", i)] = xqr[i][:].rearrange("p r c -> p (r c)")
                flat[("i", i)] = xqi[i][:].rearrange("p r c -> p (r c)")
                flat[("s", i)] = xqs[i][:].rearrange("p r c -> p (r c)")

            # ---- binarize input + build the sum frame, 14-row units ----
            # xqr, xqi = sign(x) in {-1,1} (ScalarE Sign, 3/4-row chunks);
            # xqs = xqr + xqi in {-2,0,2}, 7-row halves split GpSimd/DVE.
            def unit_dma(img, r0):
                st = stage_pool.tile([CPB, 2, UR * W], FP8E5, tag="stage")
                ch0 = img * 2 * CPB
                src = bass.AP(
                    x_flat.tensor,
                    ch0 * (H * W) + r0 * W,
                    [[H * W, CPB], [CPB * H * W, 2], [1, UR * W]],
                )
                nc.sync.dma_start(st[:], src)
                return st

            def chunk_sign(img, r0, src2d, a, b, dstf):
                rws = slice(r0 + 2 + a, r0 + 2 + b)
                nc.scalar.activation(
                    dstf[img][:, rws, 1:113],
                    src2d[:, a * W:b * W].rearrange("p (r c) -> p r c", c=W),
                    AF.Sign, bias=eps_pos[:], scale=1.0,
                )

            def unit_sign(img, r0, st):
                for cib, dstf in ((0, xqr), (1, xqi)):
                    for a, b in SIGN_CHUNKS:
                        chunk_sign(img, r0, st[:, cib, :], a, b, dstf)

            def unit_sum(img, r0, nr=7, eng=None):
                # full-width (pad cols are 0 in both operands and stay 0)
                if eng is not None:
                    r_a = slice(r0 + 2, r0 + 2 + nr)
                    eng.tensor_tensor(
                        xqs[img][:, r_a, :], xqr[img][:, r_a, :],
                        xqi[img][:, r_a, :], op=ALU.add,
                    )
                    return
                r_g = slice(r0 + 2, r0 + 2 + 4)
                r_v = slice(r0 + 2 + 4, r0 + 2 + nr)
                nc.gpsimd.tensor_tensor(
                    xqs[img][:, r_g, :], xqr[img][:, r_g, :],
                    xqi[img][:, r_g, :], op=ALU.add,
                )
                nc.vector.tensor_tensor(
                    xqs[img][:, r_v, :], xqr[img][:, r_v, :],
                    xqi[img][:, r_v, :], op=ALU.add,
                )

            unit_q = [(im, r0) for im in range(IMGS)
                      for r0 in range(0, H, UR)]

            def pop_unit():
                if unit_q:
                    im, r0 = unit_q.pop(0)
                    st = unit_dma(im, r0)
                    unit_sign(im, r0, st)
                    unit_sum(im, r0)

            def conv(img, t, kind):
                base = (4 * t + 2) * RS
                w3 = {"r": wq_r, "i": wq_i, "s": wq_s}[kind]
                xf = flat[(kind, img)]
                ps = psum_pool.tile([CPB, NT], F32, tag="ps",
                                    name=f"ps_{kind}{img}_{t}")
                part = [list(xf.ap)[0][0], CPB]
                for p in range(4):
                    o0, o1 = TAP_OFF[2 * p], TAP_OFF[2 * p + 1]
                    rhs = bass.AP(
                        xf.tensor, xf.offset + o0 + base,
                        [part, [o1 - o0, 2], [1, NT]],
                    )
                    nc.tensor.matmul(
                        ps[:], w3[:, 2 * p:2 * p + 2, :], rhs,
                        start=(p == 0), stop=False, perf_mode=DRM,
                    )
                nc.tensor.matmul(
                    ps[:], w3[:, 8, :],
                    xf[:, base + TAP_OFF[8]:base + TAP_OFF[8] + NT],
                    start=False, stop=True,
                )
                return ps

            def dma_out(img, t, osb, cib, a, b):
                # flat 448-elem runs; cib selects real/imag half, rows a..b
                dst = bass.AP(
                    out_flat.tensor,
                    (img * 2 + cib) * CPB * H * W + (4 * t + a) * W,
                    [[H * W, CPB], [1, (b - a) * W]],
                )
                nc.sync.dma_start(dst, osb[:, cib, a * W:b * W])

            # out_real = A - B + bias_r ; out_imag = C - A - B + bias_i
            # Bank evictions are spread over both PSUM-capable engines:
            #   An2 = A + bias_r (ScalarE), Bn0 = -B (DVE; ScalarE in the
            #   last band), t5 = C - An2 (DVE); out_real = An2 + Bn0
            #   (GpSimd, SBUF-only); out_imag = (t5 + (bias_r+bias_i)) +
            #   Bn0 (DVE)
            def finish_tile(img, t, A, tail=False, last=False):
                An2 = tmp_pool.tile([CPB, TROWS, W], F32, tag="An")
                Av = A[:].rearrange("p (r c) -> p r c", c=RS)
                nc.scalar.activation(An2[:], Av[:, :, 1:113], AF.Identity,
                                     bias=bias_sb[:, 0:1], scale=1.0)
                Bp = conv(img, t, "i")
                Bn0 = tmp_pool.tile([CPB, TROWS, W], F32, tag="Bn")
                Bv = Bp[:].rearrange("p (r c) -> p r c", c=RS)
                if tail:
                    # ScalarE: Bn0 = -B + eps (~1e-6 absolute error, far
                    # below the fp16 output quantum) so DVE enters the
                    # kernel tail with no backlog
                    nc.scalar.activation(Bn0[:], Bv[:, :, 1:113], AF.Identity,
                                         bias=eps_pos[:], scale=-1.0)
                else:
                    nc.vector.tensor_scalar(Bn0[:], Bv[:, :, 1:113],
                                            -1.0, None, op0=ALU.mult)
                C = conv(img, t, "s")
                Cv = C[:].rearrange("p (r c) -> p r c", c=RS)

                osb = out_pool.tile([CPB, 2, TROWS * W], F16, tag="osb")
                osb_r = osb[:, 0, :].rearrange("p (r c) -> p r c", c=W)
                nc.gpsimd.tensor_tensor(osb_r, An2[:], Bn0[:], op=ALU.add)
                if tail:
                    # real half ships as soon as it's assembled; shortens
                    # the end-of-kernel drain for the last tiles
                    dma_out(img, t, osb, 0, 0, TROWS)
                if last:
                    # split the imag chain in 2-row chunks: the final
                    # serial path after the last matmul is halved
                    for a, b in ((0, 2), (2, TROWS)):
                        t5 = tmp_pool.tile([CPB, b - a, W], F32, tag="t5")
                        nc.vector.tensor_sub(t5[:], Cv[:, a:b, 1:113],
                                             An2[:, a:b, :])
                        nc.vector.scalar_tensor_tensor(
                            osb[:, 1, a * W:b * W].rearrange(
                                "p (r c) -> p r c", c=W),
                            t5[:], bias_ir[:], Bn0[:, a:b, :],
                            op0=ALU.add, op1=ALU.add,
                        )
                        dma_out(img, t, osb, 1, a, b)
                    return
                t5 = tmp_pool.tile([CPB, TROWS, W], F32, tag="t5")
                nc.vector.tensor_sub(t5[:], Cv[:, :, 1:113], An2[:])
                nc.vector.scalar_tensor_tensor(
                    osb[:, 1, :].rearrange("p (r c) -> p r c", c=W),
                    t5[:], bias_ir[:], Bn0[:],
                    op0=ALU.add, op1=ALU.add,
                )
                if tail:
                    dma_out(img, t, osb, 1, 0, TROWS)
                else:
                    # one DMA for both channel halves: flat 448-elem runs,
                    # dst walks [ch-within-block, block, flat-rows]
                    dst = bass.AP(
                        out_flat.tensor,
                        img * 2 * CPB * H * W + 4 * t * W,
                        [[H * W, CPB], [CPB * H * W, 2], [1, TROWS * W]],
                    )
                    nc.sync.dma_start(dst, osb[:])

            gtile = [0]

            def finish_last(img, t):
                # final tile: conv order r -> s -> i so everything except
                # -B and the output assembly runs before the last matmul;
                # the imag DMA issues from the ScalarE hwdge ring to dodge
                # the sync queue
                A = conv(img, t, "r")
                An2 = tmp_pool.tile([CPB, TROWS, W], F32, tag="An")
                Av = A[:].rearrange("p (r c) -> p r c", c=RS)
                nc.scalar.activation(An2[:], Av[:, :, 1:113], AF.Identity,
                                     bias=bias_sb[:, 0:1], scale=1.0)
                C = conv(img, t, "s")
                Cv = C[:].rearrange("p (r c) -> p r c", c=RS)
                t5p = tmp_pool.tile([CPB, TROWS, W], F32, tag="t5")
                # t5p = (C + (bias_r+bias_i)) - An2, during the i-conv
                nc.vector.scalar_tensor_tensor(
                    t5p[:], Cv[:, :, 1:113], bias_ir[:], An2[:],
                    op0=ALU.add, op1=ALU.subtract,
                )
                Bp = conv(img, t, "i")
                Bv = Bp[:].rearrange("p (r c) -> p r c", c=RS)
                Bn0 = tmp_pool.tile([CPB, TROWS, W], F32, tag="Bn")
                nc.scalar.activation(Bn0[:], Bv[:, :, 1:113], AF.Identity,
                                     bias=eps_pos[:], scale=-1.0)
                osb = out_pool.tile([CPB, 2, TROWS * W], F16, tag="osb")
                osb_i = osb[:, 1, :].rearrange("p (r c) -> p r c", c=W)
                nc.vector.tensor_tensor(osb_i, t5p[:], Bn0[:], op=ALU.add)
                dst_i = bass.AP(
                    out_flat.tensor,
                    (img * 2 + 1) * CPB * H * W + 4 * t * W,
                    [[H * W, CPB], [1, TROWS * W]],
                )
                nc.scalar.dma_start(dst_i, osb[:, 1, :])
                osb_r = osb[:, 0, :].rearrange("p (r c) -> p r c", c=W)
                nc.vector.tensor_tensor(osb_r, An2[:], Bn0[:], op=ALU.add)
                dst_r = bass.AP(
                    out_flat.tensor,
                    img * 2 * CPB * H * W + 4 * t * W,
                    [[H * W, CPB], [1, TROWS * W]],
                )
                nc.scalar.dma_start(dst_r, osb[:, 0, :])

            def conv_tiles(img, tiles, stagger=0):
                # stagger: run the r-convs of the first few tiles back to
                # back so the PE has work while the i/s inputs (later on
                # the DMA ring / ScalarE queue) are still landing
                pre = {t: conv(img, t, "r") for t in tiles[:stagger]}
                for t in tiles:
                    if (img == IMGS - 1) and t == 27:
                        finish_last(img, t)
                        gtile[0] += 1
                        continue
                    A = pre.pop(t) if t in pre else conv(img, t, "r")
                    tail = (img == IMGS - 1) and t >= 20
                    finish_tile(img, t, A, tail=tail)
                    # 2 binarize units per 3 tiles interleave finely with
                    # the evac ops and stay well ahead of their consumers
                    if gtile[0] % 3 != 2:
                        pop_unit()
                    gtile[0] += 1

            ranges = [range(0, 6), range(6, 13), range(13, 20), range(20, 28)]
            groups = [(i, b) for i in range(IMGS) for b in range(H // BAND)]

            # ---- manual head, ordered by need-time on the DMA ring ----
            # ring: w_all (above) -> unit0 -> unit1 -> bias -> units 2,3.
            # ScalarE does only the unit Signs (r-halves first); the weight
            # binarize runs on DVE, the head unit sums on DVE/GpSimd.
            st0 = unit_dma(0, 0)
            st1 = unit_dma(0, 7)
            bias_sb = bias_pool.tile([CPB, 2], F32, tag="biassb")
            nc.sync.dma_start(bias_sb[:], b_ext[:])

            wtmp = wstage_pool.tile([CPB, WCOLS], FP8, tag="wtmp")
            wsign(wq_rf, wtmp, 0, 0, 768)
            for a, b in SIGN_CHUNKS:
                chunk_sign(0, 0, st0[:, 0, :], a, b, xqr)
            wsign(wq_rf, wtmp, 0, 768, WCOLS)
            for a, b in SIGN_CHUNKS:
                chunk_sign(0, 7, st1[:, 0, :], a, b, xqr)
            wsign(wq_if, wtmp, WCOLS, 0, 768)
            wsign(wq_if, wtmp, WCOLS, 768, WCOLS)
            for a, b in SIGN_CHUNKS:
                chunk_sign(0, 0, st0[:, 1, :], a, b, xqi)
            for a, b in SIGN_CHUNKS:
                chunk_sign(0, 7, st1[:, 1, :], a, b, xqi)
            nc.vector.tensor_tensor(wq_s[:], wq_r[:], wq_i[:], op=ALU.add)
            bias_ir = bias_pool.tile([CPB, 1], F32, tag="biasir")
            nc.vector.tensor_add(bias_ir[:], bias_sb[:, 1:2], bias_sb[:, 0:1])
            unit_sum(0, 0, eng=nc.vector)
            unit_sum(0, 7, eng=nc.gpsimd)
            del unit_q[:2]
            pop_unit()  # unit 2: rows 14-20
            pop_unit()  # unit 3: rows 21-27
            for gi, (img, b) in enumerate(groups):
                tiles = list(ranges[b])
                conv_tiles(img, tiles, stagger=3 if gi == 0 else 0)
                if gi == 0:
                    frame_memsets(1)

    _split_multiwait(nc)
    return nc


def _prep(x, weight_real, weight_imag, bias):
    import ml_dtypes
    e5 = ml_dtypes.float8_e5m2
    x = np.ascontiguousarray(np.asarray(x, dtype=np.float32).astype(e5))
    wr = np.asarray(weight_real, dtype=np.float32)
    wi = np.asarray(weight_imag, dtype=np.float32)
    bias = np.asarray(bias, dtype=np.float32)
    wrT = wr.transpose(1, 2, 3, 0).reshape(CPB, 9 * CPB)
    wiT = wi.transpose(1, 2, 3, 0).reshape(CPB, 9 * CPB)
    bias2 = bias.reshape(2, CPB).T
    wT = np.ascontiguousarray(np.concatenate([wrT, wiT], axis=1).astype(e5))
    bias2 = np.ascontiguousarray(bias2)
    return [
        {"x": x[IMGS * c:IMGS * (c + 1)], "wT": wT, "bias2": bias2}
        for c in range(N_CORES)
    ]


def kernel(x, weight_real, weight_imag, bias):
    in_maps = _prep(x, weight_real, weight_imag, bias)
    nc = build_nc()
    res = run_bass_kernel_spmd(nc, in_maps, core_ids=list(range(N_CORES)))
    out = np.concatenate([res.results[i]["out"] for i in range(N_CORES)], axis=0)
    return out.astype(np.float32)


def run_traced(x, weight_real, weight_imag, bias, **trace_kwargs):
    """test.py entry: same as kernel() but with neuron-profile tracing."""
    in_maps = _prep(x, weight_real, weight_imag, bias)
    nc = build_nc()
    res = run_bass_kernel_spmd(
        nc, in_maps, core_ids=list(range(N_CORES)), trace=True, **trace_kwargs
    )
    out = np.concatenate([res.results[i]["out"] for i in range(N_CORES)], axis=0)
    return out.astype(np.float32), res


# revision 36
# speedup vs baseline: 1.0050x; 1.0050x over previous
"""Binary complex conv (BC conv) on 8 TRN2 NeuronCores.

Reference computation:
    xb = sign(x + 1e-6)                      # (16, 256, 112, 112)
    wr = sign(weight_real + 1e-6)            # (128, 128, 3, 3)
    wi = sign(weight_imag + 1e-6)
    kernel = [[wr, -wi], [wi, wr]]           # (256, 256, 3, 3)
    out = conv2d(xb, kernel, pad=1) + bias   # (16, 256, 112, 112)

Strategy: pure data-parallel over batch (2 images per core); everything
else on-device, numerically exact (all matmul operands are +-1/0/+-2 ->
exact in fp8e4; PSUM accumulates fp32; fp16 output is exact for the
integer conv part, bias rounds at ~2^-10).

Two tricks on top of the direct conv:
 * Karatsuba for the complex structure: A = xr*wr, B = xi*wi,
   C = (xr+xi)*(wr+wi); out_real = A-B, out_imag = C-A-B.
   3 convs of 128 input channels instead of 4.
 * fp8 DoubleRow: each binarized frame is stored with row stride 113
   (shared pad column: each row's right pad is the next row's left
   pad); conv taps in raster order have flat offsets [-114,-113,-112,
   -1,0,1,112,113,114], so consecutive taps pair into DoubleRow
   matmuls (contraction 256) + one normal matmul per conv.
   (Measured: every matmul streams at ~194ns = the 456-column output
   rate, so the steady state IS the PE streaming bound; DoublePixel
   is uint8-only in silicon and walrus rejects integer matmuls.)

Each 4-output-row tile accumulates into a [128, 452] PSUM bank
(garbage pad lanes skipped on eviction).

I/O transport (all actual computation stays on device):
 * x and weights ship as fp8e5 raw casts of the fp32 inputs - sign(x +
   1e-6) is preserved for all but ~3e-6 of elements (measured 159 of
   51M, zero weight flips), far inside the 2e-2 tolerance. 4x less
   input DMA; the bandwidth-bound head shrinks accordingly.
 * fp16 output, shipped as flat 448-element runs: halves output HBM
   traffic and the end-of-kernel drain.

Scheduling (from trace analysis):
 * Sign act-table preloaded at t=0; weight Signs split in piece-gated
   chunks so the first conv's DoubleRow matmuls start on taps 0-5
   while taps 6-8 still binarize.
 * head DMAs batched into few issues (each DMA_DIRECT2D costs ~650ns
   of queue occupancy, which otherwise dominates the head): one
   weights+bias DMA, one 14-row r-half, one 14-row i-half.
 * binarize in 14-row units (one DMA, both halves), Signs in 3/4-row
   chunks so no engine queue holds a long op ahead of the short PSUM
   evacuations that gate bank recycling; one unit per 3 tiles.
 * engine balance: ScalarE: Sign binarize + A-bank evac; DVE: B/C-bank
   evac + imag assembly + 3/7 of sums; GpSimd: real assembly + 4/7 of
   sums. In the last band (no more binarize work) Bn0 moves to ScalarE
   so DVE+GpSimd enter the tail without backlog; the last tiles' imag
   assembly and output DMAs split in 2-row chunks to shorten the
   final serial chain.
"""

import numpy as np

import concourse.bass as bass
import concourse.tile as tile
from concourse import mybir
from concourse.bass_utils import run_bass_kernel_spmd

N_CORES = 8
B = 16
CPB = 128          # channels per block (partition dim)
H = W = 112
RS = 113           # frame row stride (shared pad column:
                   # col 112 of row y is col 0 of row y+1)
FROWS = 116        # 114 padded rows + 2 junk margin rows
IMGS = 2
TROWS = 4          # output rows per matmul tile
NT = TROWS * RS    # matmul free dim (456)
BAND = 28
UR = 7             # binarize unit rows
EPS = 1e-6
WCOLS = 9 * CPB    # 1152
WTOT = 2 * WCOLS      # wr | wi

F32 = mybir.dt.float32
F16 = mybir.dt.float16
FP8 = mybir.dt.float8e4
FP8E5 = mybir.dt.float8e5
AF = mybir.ActivationFunctionType
DRM = mybir.MatmulPerfMode.DoubleRow
ALU = mybir.AluOpType

# tap flat offsets in raster order; pairs (0,1) (2,3) (4,5) (6,7), single 8
TAP_OFF = [dy * RS + dx for dy in (-1, 0, 1) for dx in (-1, 0, 1)]

# sign-chunk row ranges within a 7-row unit
SIGN_CHUNKS = ((0, 4), (4, 7))


def _split_multiwait(nc):
    """Walrus in this container rejects >1 semaphore wait per instruction
    ("Too many sync wait commands"); hoist extra waits onto preceding nops
    on the same engine."""
    import bass_rust

    for f in nc.m.functions:
        for bb in f.blocks:
            new_insts = []
            for inst in bb.instructions:
                si = inst.sync_info
                waits = list(si.on_wait) if si is not None and si.on_wait else []
                if len(waits) > 1:
                    for w in waits[:-1]:
                        nop = mybir.InstNoOp(
                            name=nc.get_next_instruction_name(),
                            engine=inst.engine,
                            ins=[],
                            outs=[],
                        )
                        nop.sync_info = bass_rust.SyncInfo(on_wait=[w], on_update=[])
                        new_insts.append(nop)
                    si.on_wait = [waits[-1]]
                    inst.sync_info = si
                new_insts.append(inst)
            bb.instructions = new_insts


def _batch_pe_incs(nc):
    """All PE matmuls inc one progress semaphore by 1 and every consumer
    waits sem-ge on a count keyed to a matmul position. Walrus only allows
    update_value==1, so instead of batching values: strip the increments
    from non-stop matmuls and remap every wait on that semaphore to
    group-count scale (a wait for matmul #v becomes a wait for the group
    containing #v; matmuls complete in program order, so releases move to
    the group end, which is where every PSUM evacuation keys anyway)."""
    mms = []
    for f in nc.m.functions:
        for bb in f.blocks:
            for inst in bb.instructions:
                if isinstance(inst, mybir.InstMatmult):
                    mms.append(inst)
    sid = mms[0].sync_info.on_update[0].id
    for m in mms:
        ups = m.sync_info.on_update
        assert len(ups) == 1 and ups[0].id == sid and ups[0].update_value == 1
    group_of = {}
    g = 0
    for i, m in enumerate(mms, start=1):
        group_of[i] = g + 1
        if m.stop_tensor_calc:
            g += 1
    for f in nc.m.functions:
        for bb in f.blocks:
            for inst in bb.instructions:
                si = inst.sync_info
                if si is None:
                    continue
                for w in (si.on_wait or []):
                    if w.id == sid:
                        assert str(w.wait_mode) == "sem-ge-imm", w.wait_mode
                        v = w.wait_value
                        assert 1 <= v <= len(mms), v
                        w.wait_value = group_of[v]
    for m in mms:
        if not m.stop_tensor_calc:
            si = m.sync_info
            si.on_update = []
            m.sync_info = si


def build_nc():
    nc = bass.Bass()

    x_ext = nc.declare_dram_parameter("x", [IMGS, 2 * CPB, H, W], FP8E5,
                                      isOutput=False)
    w_ext = nc.declare_dram_parameter("wT", [CPB, WTOT], FP8E5, isOutput=False)
    b_ext = nc.declare_dram_parameter("bias2", [CPB, 2], F32, isOutput=False)
    out_ext = nc.declare_dram_parameter("out", [IMGS, 2 * CPB, H, W], F16,
                                        isOutput=True)

    x_flat = x_ext.rearrange("b c h w -> (b c) (h w)")     # [512, 12544]
    out_flat = out_ext.rearrange("b c h w -> (b c) (h w)")

    with tile.TileContext(nc) as tc:
        with (
            tc.tile_pool(name="wstage", bufs=1) as wstage_pool,
            tc.tile_pool(name="wbin", bufs=1) as wbin_pool,
            tc.tile_pool(name="biasp", bufs=1) as bias_pool,
            tc.tile_pool(name="xq", bufs=1) as xq_pool,
            tc.tile_pool(name="stage", bufs=6) as stage_pool,
            tc.tile_pool(name="tmp", bufs=9) as tmp_pool,
            tc.tile_pool(name="outsb", bufs=8) as out_pool,
            tc.tile_pool(name="psum", bufs=8, space="PSUM") as psum_pool,
        ):
            # per-partition scalar constant for activation bias
            eps_pos = bias_pool.tile([CPB, 1], F32, tag="epsp")
            nc.gpsimd.memset(eps_pos[:], EPS)

            # HAM warmup: dummy matmuls on junk data with no dependencies so
            # the PE clock-gate reaches 8/8 before the first real matmul.
            # Sized to end roughly when the first conv's inputs land.
            junk = bias_pool.tile([CPB, 512], FP8, tag="junk")
            nc.gpsimd.memset(junk[:, 0:1], 1.0)

            # Sign act-table preload: a tiny dependency-free activation so
            # the 1.3us ACT_TABLE_LOAD runs at t~0, not ahead of the first
            # real binarize.
            sgate = bias_pool.tile([CPB, 1], FP8, tag="sgate")
            nc.scalar.activation(sgate[:], eps_pos[:], AF.Sign,
                                 bias=eps_pos[:], scale=1.0)

            jps = psum_pool.tile([CPB, 512], F32, tag="ps", name="jps")
            for _ in range(8):
                nc.tensor.matmul(jps[:], junk[:, :CPB], junk[:], start=True,
                                 stop=True)
            for _ in range(6):
                nc.tensor.matmul(jps[:, :256], junk[:, :CPB], junk[:, :256],
                                 start=True, stop=True)
            jout = bias_pool.tile([CPB, 1], F32, tag="jout")
            nc.vector.tensor_copy(jout[:], jps[:, 0:1])

            # DMA wake: a 128B transfer ahead of the weights so the first
            # real DMA doesn't pay the cold-path latency
            wake = bias_pool.tile([CPB, 1], FP8E5, tag="wake")
            nc.sync.dma_start(wake[:], x_flat[0:CPB, 0:1])

            # ---- weights: one DMA for wr|wi|bias ----
            w_sb = wstage_pool.tile([CPB, WTOT], FP8E5, tag="wstage")
            nc.sync.dma_start(w_sb[:], w_ext[:])

            # binarized fp8 weights [ci, tap, co]; wq_s = wq_r + wq_i
            wq_r = wbin_pool.tile([CPB, 9, CPB], FP8, tag="wqr")
            wq_i = wbin_pool.tile([CPB, 9, CPB], FP8, tag="wqi")
            wq_s = wbin_pool.tile([CPB, 9, CPB], FP8, tag="wqs")
            wq_rf = wq_r[:].rearrange("p t c -> p (t c)")
            wq_if = wq_i[:].rearrange("p t c -> p (t c)")

            def wsign(dstf, tmpf, col0, a, b):
                # DVE binarize: (w >= -eps)*2 - 1 in {-1,1}; keeps the
                # weight binarize off the ScalarE queue, which the unit
                # Signs and PSUM evacs saturate in the head
                nc.vector.tensor_scalar(tmpf[:, a:b], w_sb[:, col0 + a:col0 + b],
                                        -EPS, 2.0, op0=ALU.is_ge, op1=ALU.mult)
                nc.vector.tensor_scalar(dstf[:, a:b], tmpf[:, a:b],
                                        1.0, None, op0=ALU.subtract)

            # ---- persistent binarized fp8 frames ----
            # frame: [128, FROWS, RS]; frame row = padded row + 1 (1 junk
            # margin row on top); cols 0 / 113 are the zero pad columns,
            # cols 114-115 slack (only ever read into discarded pad lanes)
            def frame(nm):
                return xq_pool.tile([CPB, FROWS, RS], FP8, tag=nm, name=nm)

            xqr = [frame(f"xqr{i}") for i in range(IMGS)]
            xqi = [frame(f"xqi{i}") for i in range(IMGS)]
            xqs = [frame(f"xqs{i}") for i in range(IMGS)]

            def frame_memsets(i):
                eng = nc.vector if i == 0 else nc.gpsimd
                for t in (xqr[i], xqi[i], xqs[i]):
                    eng.memset(t[:, 1:2, :], 0.0)          # padded row 0
                    eng.memset(t[:, 114:115, :], 0.0)      # padded row 113
                    eng.memset(t[:, 1:116, 0:1], 0.0)      # padded col 0 (+row
                   # 115's, read as row 114's shared right pad)

            frame_memsets(0)  # img1's memsets deferred past the first band

            flat = {}
            for i in range(IMGS):
                flat[("r", i)] = xqr[i][:].rearrange("p r c -> p (r c)")
                flat[("i", i)] = xqi[i][:].rearrange("p r c -> p (r c)")
                flat[("s", i)] = xqs[i][:].rearrange("p r c -> p (r c)")

            # ---- binarize input + build the sum frame, 14-row units ----
            # xqr, xqi = sign(x) in {-1,1} (ScalarE Sign, 3/4-row chunks);
            # xqs = xqr + xqi in {-2,0,2}, 7-row halves split GpSimd/DVE.
            def unit_dma(img, r0):
                st = stage_pool.tile([CPB, 2, UR * W], FP8E5, tag="stage")
                ch0 = img * 2 * CPB
                src = bass.AP(
                    x_flat.tensor,
                    ch0 * (H * W) + r0 * W,
                    [[H * W, CPB], [CPB * H * W, 2], [1, UR * W]],
                )
                nc.sync.dma_start(st[:], src)
                return st

            def chunk_sign(img, r0, src2d, a, b, dstf):
                rws = slice(r0 + 2 + a, r0 + 2 + b)
                nc.scalar.activation(
                    dstf[img][:, rws, 1:113],
                    src2d[:, a * W:b * W].rearrange("p (r c) -> p r c", c=W),
                    AF.Sign, bias=eps_pos[:], scale=1.0,
                )

            def unit_sign(img, r0, st):
                for cib, dstf in ((0, xqr), (1, xqi)):
                    for a, b in SIGN_CHUNKS:
                        chunk_sign(img, r0, st[:, cib, :], a, b, dstf)

            def unit_sum(img, r0, nr=7, eng=None):
                # full-width (pad cols are 0 in both operands and stay 0)
                if eng is not None:
                    r_a = slice(r0 + 2, r0 + 2 + nr)
                    eng.tensor_tensor(
                        xqs[img][:, r_a, :], xqr[img][:, r_a, :],
                        xqi[img][:, r_a, :], op=ALU.add,
                    )
                    return
                r_g = slice(r0 + 2, r0 + 2 + 4)
                r_v = slice(r0 + 2 + 4, r0 + 2 + nr)
                nc.gpsimd.tensor_tensor(
                    xqs[img][:, r_g, :], xqr[img][:, r_g, :],
                    xqi[img][:, r_g, :], op=ALU.add,
                )
                nc.vector.tensor_tensor(
                    xqs[img][:, r_v, :], xqr[img][:, r_v, :],
                    xqi[img][:, r_v, :], op=ALU.add,
                )

            unit_q = [(im, r0) for im in range(IMGS)
                      for r0 in range(0, H, UR)]

            def pop_unit():
                if unit_q:
                    im, r0 = unit_q.pop(0)
                    st = unit_dma(im, r0)
                    unit_sign(im, r0, st)
                    unit_sum(im, r0)

            def conv(img, t, kind):
                base = (4 * t + 2) * RS
                w3 = {"r": wq_r, "i": wq_i, "s": wq_s}[kind]
                xf = flat[(kind, img)]
                ps = psum_pool.tile([CPB, NT], F32, tag="ps",
                                    name=f"ps_{kind}{img}_{t}")
                part = [list(xf.ap)[0][0], CPB]
                for p in range(4):
                    o0, o1 = TAP_OFF[2 * p], TAP_OFF[2 * p + 1]
                    rhs = bass.AP(
                        xf.tensor, xf.offset + o0 + base,
                        [part, [o1 - o0, 2], [1, NT]],
                    )
                    nc.tensor.matmul(
                        ps[:], w3[:, 2 * p:2 * p + 2, :], rhs,
                        start=(p == 0), stop=False, perf_mode=DRM,
                    )
                nc.tensor.matmul(
                    ps[:], w3[:, 8, :],
                    xf[:, base + TAP_OFF[8]:base + TAP_OFF[8] + NT],
                    start=False, stop=True,
                )
                return ps

            def dma_out(img, t, osb, cib, a, b):
                # flat 448-elem runs; cib selects real/imag half, rows a..b
                dst = bass.AP(
                    out_flat.tensor,
                    (img * 2 + cib) * CPB * H * W + (4 * t + a) * W,
                    [[H * W, CPB], [1, (b - a) * W]],
                )
                nc.sync.dma_start(dst, osb[:, cib, a * W:b * W])

            # out_real = A - B + bias_r ; out_imag = C - A - B + bias_i
            # Bank evictions are spread over both PSUM-capable engines:
            #   An2 = A + bias_r (ScalarE), Bn0 = -B (DVE; ScalarE in the
            #   last band), t5 = C - An2 (DVE); out_real = An2 + Bn0
            #   (GpSimd, SBUF-only); out_imag = (t5 + (bias_r+bias_i)) +
            #   Bn0 (DVE)
            def finish_tile(img, t, A, tail=False, last=False):
                An2 = tmp_pool.tile([CPB, TROWS, W], F32, tag="An")
                Av = A[:].rearrange("p (r c) -> p r c", c=RS)
                nc.scalar.activation(An2[:], Av[:, :, 1:113], AF.Identity,
                                     bias=bias_sb[:, 0:1], scale=1.0)
                Bp = conv(img, t, "i")
                Bn0 = tmp_pool.tile([CPB, TROWS, W], F32, tag="Bn")
                Bv = Bp[:].rearrange("p (r c) -> p r c", c=RS)
                if tail:
                    # ScalarE: Bn0 = -B + eps (~1e-6 absolute error, far
                    # below the fp16 output quantum) so DVE enters the
                    # kernel tail with no backlog
                    nc.scalar.activation(Bn0[:], Bv[:, :, 1:113], AF.Identity,
                                         bias=eps_pos[:], scale=-1.0)
                else:
                    nc.vector.tensor_scalar(Bn0[:], Bv[:, :, 1:113],
                                            -1.0, None, op0=ALU.mult)
                C = conv(img, t, "s")
                Cv = C[:].rearrange("p (r c) -> p r c", c=RS)

                osb = out_pool.tile([CPB, 2, TROWS * W], F16, tag="osb")
                osb_r = osb[:, 0, :].rearrange("p (r c) -> p r c", c=W)
                nc.gpsimd.tensor_tensor(osb_r, An2[:], Bn0[:], op=ALU.add)
                if tail:
                    # real half ships as soon as it's assembled; shortens
                    # the end-of-kernel drain for the last tiles
                    dma_out(img, t, osb, 0, 0, TROWS)
                if last:
                    # split the imag chain in 2-row chunks: the final
                    # serial path after the last matmul is halved
                    for a, b in ((0, 2), (2, TROWS)):
                        t5 = tmp_pool.tile([CPB, b - a, W], F32, tag="t5")
                        nc.vector.tensor_sub(t5[:], Cv[:, a:b, 1:113],
                                             An2[:, a:b, :])
                        nc.vector.scalar_tensor_tensor(
                            osb[:, 1, a * W:b * W].rearrange(
                                "p (r c) -> p r c", c=W),
                            t5[:], bias_ir[:], Bn0[:, a:b, :],
                            op0=ALU.add, op1=ALU.add,
                        )
                        dma_out(img, t, osb, 1, a, b)
                    return
                t5 = tmp_pool.tile([CPB, TROWS, W], F32, tag="t5")
                nc.vector.tensor_sub(t5[:], Cv[:, :, 1:113], An2[:])
                nc.vector.scalar_tensor_tensor(
                    osb[:, 1, :].rearrange("p (r c) -> p r c", c=W),
                    t5[:], bias_ir[:], Bn0[:],
                    op0=ALU.add, op1=ALU.add,
                )
                if tail:
                    dma_out(img, t, osb, 1, 0, TROWS)
                else:
                    # one DMA for both channel halves: flat 448-elem runs,
                    # dst walks [ch-within-block, block, flat-rows]
                    dst = bass.AP(
                        out_flat.tensor,
                        img * 2 * CPB * H * W + 4 * t * W,
                        [[H * W, CPB], [CPB * H * W, 2], [1, TROWS * W]],
                    )
                    nc.sync.dma_start(dst, osb[:])

            gtile = [0]

            def finish_last(img, t):
                # final tile: conv order r -> s -> i so everything except
                # -B and the output assembly runs before the last matmul;
                # the imag DMA issues from the ScalarE hwdge ring to dodge
                # the sync queue
                A = conv(img, t, "r")
                An2 = tmp_pool.tile([CPB, TROWS, W], F32, tag="An")
                Av = A[:].rearrange("p (r c) -> p r c", c=RS)
                nc.scalar.activation(An2[:], Av[:, :, 1:113], AF.Identity,
                                     bias=bias_sb[:, 0:1], scale=1.0)
                C = conv(img, t, "s")
                Cv = C[:].rearrange("p (r c) -> p r c", c=RS)
                t5p = tmp_pool.tile([CPB, TROWS, W], F32, tag="t5")
                # t5p = (C + (bias_r+bias_i)) - An2, during the i-conv
                nc.vector.scalar_tensor_tensor(
                    t5p[:], Cv[:, :, 1:113], bias_ir[:], An2[:],
                    op0=ALU.add, op1=ALU.subtract,
                )
                Bp = conv(img, t, "i")
                Bv = Bp[:].rearrange("p (r c) -> p r c", c=RS)
                Bn0 = tmp_pool.tile([CPB, TROWS, W], F32, tag="Bn")
                nc.scalar.activation(Bn0[:], Bv[:, :, 1:113], AF.Identity,
                                     bias=eps_pos[:], scale=-1.0)
                osb = out_pool.tile([CPB, 2, TROWS * W], F16, tag="osb")
                osb_i = osb[:, 1, :].rearrange("p (r c) -> p r c", c=W)
                nc.vector.tensor_tensor(osb_i, t5p[:], Bn0[:], op=ALU.add)
                dst_i = bass.AP(
                    out_flat.tensor,
                    (img * 2 + 1) * CPB * H * W + 4 * t * W,
                    [[H * W, CPB], [1, TROWS * W]],
                )
                nc.scalar.dma_start(dst_i, osb[:, 1, :])
                osb_r = osb[:, 0, :].rearrange("p (r c) -> p r c", c=W)
                nc.vector.tensor_tensor(osb_r, An2[:], Bn0[:], op=ALU.add)
                dst_r = bass.AP(
                    out_flat.tensor,
                    img * 2 * CPB * H * W + 4 * t * W,
                    [[H * W, CPB], [1, TROWS * W]],
                )
                nc.scalar.dma_start(dst_r, osb[:, 0, :])

            def conv_tiles(img, tiles, stagger=0):
                # stagger: run the r-convs of the first few tiles back to
                # back so the PE has work while the i/s inputs (later on
                # the DMA ring / ScalarE queue) are still landing
                pre = {t: conv(img, t, "r") for t in tiles[:stagger]}
                for t in tiles:
                    if (img == IMGS - 1) and t == 27:
                        finish_last(img, t)
                        gtile[0] += 1
                        continue
                    A = pre.pop(t) if t in pre else conv(img, t, "r")
                    tail = (img == IMGS - 1) and t >= 20
                    finish_tile(img, t, A, tail=tail)
                    # 2 binarize units per 3 tiles interleave finely with
                    # the evac ops and stay well ahead of their consumers
                    if gtile[0] % 3 != 2:
                        pop_unit()
                    gtile[0] += 1

            ranges = [range(0, 6), range(6, 13), range(13, 20), range(20, 28)]
            groups = [(i, b) for i in range(IMGS) for b in range(H // BAND)]

            # ---- manual head, ordered by need-time on the DMA ring ----
            # ring: w_all (above) -> unit0 -> unit1 -> bias -> units 2,3.
            # ScalarE does only the unit Signs (r-halves first); the weight
            # binarize runs on DVE, the head unit sums on DVE/GpSimd.
            st0 = unit_dma(0, 0)
            st1 = unit_dma(0, 7)
            bias_sb = bias_pool.tile([CPB, 2], F32, tag="biassb")
            nc.sync.dma_start(bias_sb[:], b_ext[:])

            wtmp = wstage_pool.tile([CPB, WCOLS], FP8, tag="wtmp")
            wsign(wq_rf, wtmp, 0, 0, 768)
            for a, b in SIGN_CHUNKS:
                chunk_sign(0, 0, st0[:, 0, :], a, b, xqr)
            wsign(wq_rf, wtmp, 0, 768, WCOLS)
            for a, b in SIGN_CHUNKS:
                chunk_sign(0, 7, st1[:, 0, :], a, b, xqr)
            wsign(wq_if, wtmp, WCOLS, 0, 768)
            wsign(wq_if, wtmp, WCOLS, 768, WCOLS)
            for a, b in SIGN_CHUNKS:
                chunk_sign(0, 0, st0[:, 1, :], a, b, xqi)
            for a, b in SIGN_CHUNKS:
                chunk_sign(0, 7, st1[:, 1, :], a, b, xqi)
            nc.vector.tensor_tensor(wq_s[:], wq_r[:], wq_i[:], op=ALU.add)
            bias_ir = bias_pool.tile([CPB, 1], F32, tag="biasir")
            nc.vector.tensor_add(bias_ir[:], bias_sb[:, 1:2], bias_sb[:, 0:1])
            unit_sum(0, 0, eng=nc.vector)
            unit_sum(0, 7, eng=nc.gpsimd)
            del unit_q[:2]
            pop_unit()  # unit 2: rows 14-20
            pop_unit()  # unit 3: rows 21-27
            for gi, (img, b) in enumerate(groups):
                tiles = list(ranges[b])
                conv_tiles(img, tiles, stagger=3 if gi == 0 else 0)
                if gi == 0:
                    frame_memsets(1)

    _split_multiwait(nc)
    _batch_pe_incs(nc)
    return nc


def _prep(x, weight_real, weight_imag, bias):
    import ml_dtypes
    e5 = ml_dtypes.float8_e5m2
    x = np.ascontiguousarray(np.asarray(x, dtype=np.float32).astype(e5))
    wr = np.asarray(weight_real, dtype=np.float32)
    wi = np.asarray(weight_imag, dtype=np.float32)
    bias = np.asarray(bias, dtype=np.float32)
    wrT = wr.transpose(1, 2, 3, 0).reshape(CPB, 9 * CPB)
    wiT = wi.transpose(1, 2, 3, 0).reshape(CPB, 9 * CPB)
    bias2 = bias.reshape(2, CPB).T
    wT = np.ascontiguousarray(np.concatenate([wrT, wiT], axis=1).astype(e5))
    bias2 = np.ascontiguousarray(bias2)
    return [
        {"x": x[IMGS * c:IMGS * (c + 1)], "wT": wT, "bias2": bias2}
        for c in range(N_CORES)
    ]


def kernel(x, weight_real, weight_imag, bias):
    in_maps = _prep(x, weight_real, weight_imag, bias)
    nc = build_nc()
    res = run_bass_kernel_spmd(nc, in_maps, core_ids=list(range(N_CORES)))
    out = np.concatenate([res.results[i]["out"] for i in range(N_CORES)], axis=0)
    return out.astype(np.float32)


def run_traced(x, weight_real, weight_imag, bias, **trace_kwargs):
    """test.py entry: same as kernel() but with neuron-profile tracing."""
    in_maps = _prep(x, weight_real, weight_imag, bias)
    nc = build_nc()
    res = run_bass_kernel_spmd(
        nc, in_maps, core_ids=list(range(N_CORES)), trace=True, **trace_kwargs
    )
    out = np.concatenate([res.results[i]["out"] for i in range(N_CORES)], axis=0)
    return out.astype(np.float32), res
